# revision 2
# baseline (speedup 1.0000x reference)
"""Cumulative LayerNorm (B=4, C=512, T=32000) on 8 Trainium2 NeuronCores.

Sharding: core j handles batch b = j//2, T-half h = j%2 (t in [h*16000, (h+1)*16000)).
Cumulative stats over (C, 0..t) need a carry from the first T-half; each
h=1 core recomputes it from a prefix reduce pass (no cross-core comm).
SPMD symmetry: h=0 cores run the same prefix pass on dummy data and
multiply the carry by flag=0.

Per-core pipeline (single pass over its own chunk):
  - load x tile [128, 4cb, 320t]
  - bf16 convert + bf16 square (ScalarE)
  - per-t channel sums s,q via ones-matmul into PSUM [128bcast, 320] (PE, bf16)
  - running cumsum via tensor_tensor_scan on PSUM row, chained tile-to-tile (VectorE)
  - per-segment (640 t): reshape rows to t-major [128,5] via DRAM bounce,
    finalize stats (mean/var/rsqrt) there, reshape back (f32r rounded)
  - broadcast inv/-mean*inv rows to [128,320] via K=1 f32r matmuls into PSUM (PE)
  - y = x*A + B: two tensor_tensor ops reading PSUM with cb-repeat APs (VectorE)
  - store
"""
import numpy as np

import concourse.bass as bass
import concourse.bacc as bacc
import concourse.tile as tile
from concourse import mybir
from concourse.bass_utils import run_bass_kernel_spmd

F32 = mybir.dt.float32
F32R = mybir.dt.float32r
BF16 = mybir.dt.bfloat16

B, C, T = 4, 512, 32000
NCORES = 8
TH = T // 2          # 16000 per core
CB = C // 128        # 4 channel blocks
TT = 320             # t-tile (PSUM [128, 320] f32 fits a bank; N>=256 for f32r)
SEG = 640            # segment = 128 * F
F = SEG // 128       # 5 (t-major free dim)
NSEG = TH // SEG     # 25
TPS = SEG // TT      # 2 tiles per segment
NTILE = TH // TT     # 50
EPS = 1e-08

_CACHE = {}


def _build(wb_general: bool):
    nc = bacc.Bacc()

    xc_e = nc.declare_dram_parameter("xc", [C, TH], F32, isOutput=False)
    xp_e = nc.declare_dram_parameter("xp", [C, TH], F32, isOutput=False)
    flag_e = nc.declare_dram_parameter("flag", [1, 1], F32, isOutput=False)
    invn_e = nc.declare_dram_parameter("invn", [128, F * NSEG], F32, isOutput=False)
    w_e = nc.declare_dram_parameter("w", [1, C], F32, isOutput=False)
    b_e = nc.declare_dram_parameter("b", [1, C], F32, isOutput=False)
    y_e = nc.declare_dram_parameter("y", [C, TH], F32, isOutput=True)

    xc_r = xc_e.rearrange("(cb p) t -> cb p t", p=128)
    xp_r = xp_e.rearrange("(cb p) t -> cb p t", p=128)
    y_r = y_e.rearrange("(cb p) t -> cb p t", p=128)

    with tile.TileContext(nc) as tc:
        with (
            tc.tile_pool(name="misc", bufs=1) as misc,
            tc.tile_pool(name="xin", bufs=6) as xin,
            tc.tile_pool(name="bfp", bufs=3) as bfp,
            tc.tile_pool(name="rows", bufs=3) as rows,
            tc.tile_pool(name="tmaj", bufs=3) as tmaj,
            tc.tile_pool(name="outp", bufs=4) as outp,
            tc.tile_pool(name="dram", bufs=4, space="DRAM") as dram,
        ):
            # ---- constants
            ones_bf = misc.tile([128, 128], BF16, tag="ones_bf")
            nc.vector.memset(ones_bf, 1.0)
            ones_f = misc.tile([1, 128], F32, tag="ones_f")
            nc.vector.memset(ones_f, 1.0)
            nones_f = misc.tile([1, 128], F32, tag="nones_f")
            nc.vector.memset(nones_f, -1.0)
            ones_r = misc.tile([1, 128], F32R, tag="ones_r")
            nc.scalar.copy(out=ones_r, in_=ones_f)
            nones_r = misc.tile([1, 128], F32R, tag="nones_r")
            nc.scalar.copy(out=nones_r, in_=nones_f)
            eps_t = misc.tile([128, 1], F32, tag="eps_t")
            nc.vector.memset(eps_t, EPS)
            zrow = misc.tile([1, TT], F32, tag="zrow")
            nc.vector.memset(zrow, 0.0)
            flag_t = misc.tile([1, 1], F32, tag="flag_t")
            nc.sync.dma_start(out=flag_t, in_=flag_e[:, :])
            invn_t = misc.tile([128, F * NSEG], F32, tag="invn_t")
            nc.sync.dma_start(out=invn_t, in_=invn_e[:, :])
            carry_s = misc.tile([1, 1], F32, tag="carry_s")
            carry_q = misc.tile([1, 1], F32, tag="carry_q")
            if wb_general:
                wcol = misc.tile([128, CB], F32, tag="wcol")
                bcol = misc.tile([128, CB], F32, tag="bcol")
                for cb in range(CB):
                    nc.sync.dma_start(
                        out=wcol[:, cb : cb + 1],
                        in_=w_e[0:1, cb * 128 : (cb + 1) * 128].rearrange(
                            "one p -> (one p) 1"
                        ),
                    )
                    nc.sync.dma_start(
                        out=bcol[:, cb : cb + 1],
                        in_=b_e[0:1, cb * 128 : (cb + 1) * 128].rearrange(
                            "one p -> (one p) 1"
                        ),
                    )
            else:
                # still consume w/b so the params exist on the NEFF
                wdummy = misc.tile([1, C], F32, tag="wdummy")
                nc.sync.dma_start(out=wdummy, in_=w_e[:, :])
                nc.sync.dma_start(out=wdummy, in_=b_e[:, :])

            # ---- prefix reduce phase (totals of xp, flag-gated)
            with tc.tile_pool(name="ppre", bufs=1, space="PSUM") as pre_ps:
                stot = pre_ps.tile([128, TT], F32, tag="stot")
                qtot = pre_ps.tile([128, TT], F32, tag="qtot")
                for it in range(NTILE):
                    t0 = it * TT
                    xt = xin.tile([128, CB, TT], F32, tag="xpre")
                    for cb in range(CB):
                        nc.sync.dma_start(
                            out=xt[:, cb, :], in_=xp_r[cb, :, t0 : t0 + TT]
                        )
                    xbf = bfp.tile([128, CB, TT], BF16, tag="xbf_pre")
                    nc.scalar.copy(
                        out=xbf.rearrange("p cb t -> p (cb t)"),
                        in_=xt.rearrange("p cb t -> p (cb t)"),
                    )
                    zbf = bfp.tile([128, CB, TT], BF16, tag="zbf_pre")
                    nc.scalar.square(
                        out=zbf.rearrange("p cb t -> p (cb t)"),
                        in_=xt.rearrange("p cb t -> p (cb t)"),
                    )
                    for cb in range(CB):
                        first = it == 0 and cb == 0
                        last = it == NTILE - 1 and cb == CB - 1
                        nc.tensor.matmul(
                            out=stot, lhsT=ones_bf, rhs=xbf[:, cb, :],
                            start=first, stop=last,
                        )
                        nc.tensor.matmul(
                            out=qtot, lhsT=ones_bf, rhs=zbf[:, cb, :],
                            start=first, stop=last,
                        )
                sred = misc.tile([1, 1], F32, tag="sred")
                qred = misc.tile([1, 1], F32, tag="qred")
                nc.vector.reduce_sum(
                    out=sred, in_=stot[0:1, :], axis=mybir.AxisListType.X
                )
                nc.vector.reduce_sum(
                    out=qred, in_=qtot[0:1, :], axis=mybir.AxisListType.X
                )
                nc.vector.tensor_mul(out=carry_s, in0=sred, in1=flag_t)
                nc.vector.tensor_mul(out=carry_q, in0=qred, in1=flag_t)

            # ---- main phase
            with (
                tc.tile_pool(name="pstat", bufs=2, space="PSUM") as pstat,
                tc.tile_pool(name="pab", bufs=2, space="PSUM") as pab,
            ):
                srow_tiles = []
                qrow_tiles = []
                for s in range(NSEG):
                    srow = rows.tile([1, SEG], F32, tag="srow")
                    qrow = rows.tile([1, SEG], F32, tag="qrow")
                    srow_tiles.append(srow)
                    qrow_tiles.append(qrow)
                    xts = []
                    for j in range(TPS):
                        it = s * TPS + j
                        t0 = it * TT
                        xt = xin.tile([128, CB, TT], F32, tag="x")
                        xts.append(xt)
                        for cb in range(CB):
                            nc.sync.dma_start(
                                out=xt[:, cb, :], in_=xc_r[cb, :, t0 : t0 + TT]
                            )
                        xbf = bfp.tile([128, CB, TT], BF16, tag="xbf")
                        nc.scalar.copy(
                            out=xbf.rearrange("p cb t -> p (cb t)"),
                            in_=xt.rearrange("p cb t -> p (cb t)"),
                        )
                        zbf = bfp.tile([128, CB, TT], BF16, tag="zbf")
                        nc.scalar.square(
                            out=zbf.rearrange("p cb t -> p (cb t)"),
                            in_=xt.rearrange("p cb t -> p (cb t)"),
                        )
                        ps_s = pstat.tile([128, TT], F32, tag="ps_s")
                        ps_q = pstat.tile([128, TT], F32, tag="ps_q")
                        for cb in range(CB):
                            nc.tensor.matmul(
                                out=ps_s, lhsT=ones_bf, rhs=xbf[:, cb, :],
                                start=cb == 0, stop=cb == CB - 1,
                            )
                            nc.tensor.matmul(
                                out=ps_q, lhsT=ones_bf, rhs=zbf[:, cb, :],
                                start=cb == 0, stop=cb == CB - 1,
                            )
                        # chained cumsum along t
                        if it == 0:
                            init_s, init_q = carry_s, carry_q
                        elif j == 0:
                            init_s = srow_tiles[s - 1][0:1, SEG - 1 : SEG]
                            init_q = qrow_tiles[s - 1][0:1, SEG - 1 : SEG]
                        else:
                            init_s = srow[0:1, j * TT - 1 : j * TT]
                            init_q = qrow[0:1, j * TT - 1 : j * TT]
                        nc.vector.tensor_tensor_scan(
                            out=srow[0:1, j * TT : (j + 1) * TT],
                            data0=ps_s[0:1, :], data1=zrow, initial=init_s,
                            op0=mybir.AluOpType.add, op1=mybir.AluOpType.bypass,
                        )
                        nc.vector.tensor_tensor_scan(
                            out=qrow[0:1, j * TT : (j + 1) * TT],
                            data0=ps_q[0:1, :], data1=zrow, initial=init_q,
                            op0=mybir.AluOpType.add, op1=mybir.AluOpType.bypass,
                        )

                    # ---- segment finalize in t-major [128, F]
                    sdram = dram.tile([SEG], F32, tag="sdram")
                    qdram = dram.tile([SEG], F32, tag="qdram")
                    nc.sync.dma_start(out=sdram, in_=srow)
                    nc.sync.dma_start(out=qdram, in_=qrow)
                    s_tm = tmaj.tile([128, F], F32, tag="s_tm")
                    q_tm = tmaj.tile([128, F], F32, tag="q_tm")
                    nc.sync.dma_start(
                        out=s_tm, in_=sdram.rearrange("(p f) -> p f", p=128)
                    )
                    nc.sync.dma_start(
                        out=q_tm, in_=qdram.rearrange("(p f) -> p f", p=128)
                    )
                    invn_s = invn_t[:, s * F : (s + 1) * F]
                    mean = tmaj.tile([128, F], F32, tag="mean")
                    nc.vector.tensor_mul(out=mean, in0=s_tm, in1=invn_s)
                    e2 = tmaj.tile([128, F], F32, tag="e2")
                    nc.vector.tensor_mul(out=e2, in0=q_tm, in1=invn_s)
                    msq = tmaj.tile([128, F], F32, tag="msq")
                    nc.vector.tensor_mul(out=msq, in0=mean, in1=mean)
                    var = tmaj.tile([128, F], F32, tag="var")
                    nc.vector.tensor_sub(out=var, in0=e2, in1=msq)
                    nc.vector.tensor_scalar_max(out=var, in0=var, scalar1=0.0)
                    sd = tmaj.tile([128, F], F32, tag="sd")
                    nc.scalar.activation(
                        out=sd, in_=var, func=mybir.ActivationFunctionType.Sqrt,
                        bias=eps_t, scale=1.0,
                    )
                    inv_tm = tmaj.tile([128, F], F32R, tag="inv_tm")
                    minv_tm = tmaj.tile([128, F], F32R, tag="minv_tm")
                    with nc.allow_low_precision(
                        reason="f32r rounding feeds PE broadcast matmuls"
                    ):
                        nc.vector.reciprocal(out=inv_tm, in_=sd)
                        nc.vector.tensor_mul(out=minv_tm, in0=mean, in1=inv_tm)

                    invdram = dram.tile([SEG], F32R, tag="invdram")
                    minvdram = dram.tile([SEG], F32R, tag="minvdram")
                    nc.sync.dma_start(
                        out=invdram.rearrange("(p f) -> p f", p=128), in_=inv_tm
                    )
                    nc.sync.dma_start(
                        out=minvdram.rearrange("(p f) -> p f", p=128), in_=minv_tm
                    )
                    invrow = rows.tile([1, SEG], F32R, tag="invrow")
                    minvrow = rows.tile([1, SEG], F32R, tag="minvrow")
                    nc.sync.dma_start(out=invrow, in_=invdram)
                    nc.sync.dma_start(out=minvrow, in_=minvdram)

                    # ---- normalize the segment's tiles
                    for j in range(TPS):
                        it = s * TPS + j
                        t0 = it * TT
                        xt = xts[j]
                        ps_a = pab.tile([128, TT], F32, tag="ps_a")
                        ps_b = pab.tile([128, TT], F32, tag="ps_b")
                        nc.tensor.matmul(
                            out=ps_a, lhsT=ones_r,
                            rhs=invrow[0:1, j * TT : (j + 1) * TT],
                            start=True, stop=True,
                        )
                        nc.tensor.matmul(
                            out=ps_b, lhsT=nones_r,
                            rhs=minvrow[0:1, j * TT : (j + 1) * TT],
                            start=True, stop=True,
                        )
                        rep_a = bass.AP(
                            tensor=ps_a.tensor, offset=ps_a.offset,
                            ap=[ps_a.ap[0], [0, CB], ps_a.ap[1]],
                        )
                        rep_b = bass.AP(
                            tensor=ps_b.tensor, offset=ps_b.offset,
                            ap=[ps_b.ap[0], [0, CB], ps_b.ap[1]],
                        )
                        yt = outp.tile([128, CB, TT], F32, tag="yt")
                        nc.vector.tensor_mul(out=yt, in0=xt, in1=rep_a)
                        nc.vector.tensor_add(out=yt, in0=yt, in1=rep_b)
                        if wb_general:
                            for cb in range(CB):
                                nc.scalar.activation(
                                    out=yt[:, cb, :], in_=yt[:, cb, :],
                                    func=mybir.ActivationFunctionType.Copy,
                                    bias=0.0, scale=wcol[:, cb : cb + 1],
                                )
                                nc.vector.tensor_scalar_add(
                                    out=yt[:, cb, :], in0=yt[:, cb, :],
                                    scalar1=bcol[:, cb : cb + 1],
                                )
                        for cb in range(CB):
                            nc.sync.dma_start(
                                out=y_r[cb, :, t0 : t0 + TT], in_=yt[:, cb, :]
                            )

    nc.finalize()
    return nc


def _get_kernel(wb_general: bool):
    if wb_general not in _CACHE:
        _CACHE[wb_general] = _build(wb_general)
    return _CACHE[wb_general]


def _make_in_maps(x, weight, bias):
    wb_general = not (np.all(weight == 1.0) and np.all(bias == 0.0))
    w_row = np.ascontiguousarray(weight.reshape(1, C).astype(np.float32))
    b_row = np.ascontiguousarray(bias.reshape(1, C).astype(np.float32))
    in_maps = []
    for core in range(NCORES):
        b_idx, h = core // 2, core % 2
        xc = np.ascontiguousarray(x[b_idx, :, h * TH : (h + 1) * TH])
        xp = np.ascontiguousarray(x[b_idx, :, 0:TH]) if h == 1 else xc
        flag = np.full((1, 1), float(h), np.float32)
        # invn[p, s*F + f] = 1 / (C * (h*TH + s*SEG + p*F + f + 1))
        t_local = (
            np.arange(NSEG)[:, None, None] * SEG
            + np.arange(128)[None, :, None] * F
            + np.arange(F)[None, None, :]
        )
        t_global = h * TH + t_local  # [NSEG, 128, F]
        invn = (1.0 / (C * (t_global.astype(np.float64) + 1.0))).astype(np.float32)
        invn = np.ascontiguousarray(invn.transpose(1, 0, 2).reshape(128, NSEG * F))
        in_maps.append(
            {
                "xc": xc, "xp": xp, "flag": flag, "invn": invn,
                "w": w_row, "b": b_row,
            }
        )
    return in_maps, wb_general


def kernel(x, weight, bias, _trace=False):
    x = np.asarray(x, np.float32)
    weight = np.asarray(weight, np.float32)
    bias = np.asarray(bias, np.float32)
    in_maps, wb_general = _make_in_maps(x, weight, bias)
    nc = _get_kernel(wb_general)
    res = run_bass_kernel_spmd(nc, in_maps, list(range(NCORES)), trace=_trace)
    y = np.empty((B, C, T), np.float32)
    for core in range(NCORES):
        b_idx, h = core // 2, core % 2
        y[b_idx, :, h * TH : (h + 1) * TH] = res.results[core]["y"]
    if _trace:
        return y, res
    return y
